# revision 10
# baseline (speedup 1.0000x reference)
"""Trainium2 Bass kernel for CombinedRepeatCausalLinear (parallel forward).

Computes out[b,e,t] = sum_s x[b,e,s] * W[s,t] + bias[t] where
  W[s,t] = mask(t>=s) * (w0[s]*d0^(t-s) + w1[t]*d1^(t-s))
for S = 2048, x of shape (8, 1024, 2048) fp32.

Strategy (8 NeuronCores, data-parallel over batch; fp16 datapath):
  W is causal-masked rank-2.  Split s/t into 17 chunks of C=126.  For
  target chunk J the contribution of all s < 126J is exactly rank 2:
     out[t in J] = (diag block) + d0^tl * A0_J + w1[t] d1^tl * A1_J
  with A0_J[e] = sum_{s<126J} w0[s] d0^(126J-s) x[s,e]  (A1 analogous).
  C=126 leaves 2 spare K-rows, so the cross term folds into the SAME
  K=128 matmul as the 126x126 diagonal block: moving-operand partitions
  0/1 carry the per-chunk A rows, partitions 2..127 carry the x chunk;
  the stationary's rows 0/1 are the decay rows, generated together with
  the diag block by one K=2 matmul + mask.

  Pipeline layout (v2): x arrives in 9 paired chunk DMAs on two HWDGE
  rings; per-chunk A-contribution matmuls (4-strip col-tiled PSUM
  accumulation) run as chunks land, hidden under the load.  A_1..A_15
  need only chunks 0..14, so the strip reduce + scatter for them is
  emitted BETWEEN chunk 14's and chunk 15's A matmuls (two-pass); A_16
  is extracted by a tiny second pass.  The scatter into xg rows 0/1 is
  2 DMAs via a q-major reduce layout.  Mains stream per chunk with
  PSUM->SBUF bias copies split ACT/DVE and paired 516KB output DMAs on
  the sync/gpsimd rings (scalar ring stays free for ACT copies).
  fp16 everywhere in SBUF; fp32 PSUM accumulate.
"""

import numpy as np

import concourse.bass as bass
import concourse.mybir as mybir
import concourse.tile as tile
from concourse import bacc
from concourse.bass_utils import run_bass_kernel_spmd

F16 = mybir.dt.float16
F32 = mybir.dt.float32

B = 8
E = 1024
S = 2048
DC = 1.0
N_CORES = 8
R = (B * E) // N_CORES      # rows (e) per core = 1024
C = 126                     # chunk size along s/t
NCH = 17                    # chunks; chunk 16 has only 32 valid rows
LAST = S - C * (NCH - 1)    # 32

_PROGRAM = None


def _build_program():
    nc = bacc.Bacc("TRN2", target_bir_lowering=False, debug=False,
                   num_devices=N_CORES)

    xg_d = nc.declare_dram_parameter("xg", [128, NCH * R], F16, isOutput=False)
    uu_d = nc.declare_dram_parameter("uu", [128, 16 * 32], F16,
                                     isOutput=False)
    fs_d = nc.declare_dram_parameter("fs", [2, NCH * 128], F16,
                                     isOutput=False)
    fm_d = nc.declare_dram_parameter("fm", [2, NCH * C], F16, isOutput=False)
    mask_d = nc.declare_dram_parameter("mask", [128, C], F16, isOutput=False)
    predq_d = nc.declare_dram_parameter("predq", [128, 32], F16,
                                        isOutput=False)
    predb_d = nc.declare_dram_parameter("predb", [128, 2], F16,
                                        isOutput=False)
    biasT_d = nc.declare_dram_parameter("biasT", [C, NCH], F32,
                                        isOutput=False)
    outg_d = nc.declare_dram_parameter("outg", [C, NCH * R], F16,
                                       isOutput=True)

    Ident = mybir.ActivationFunctionType.Identity

    with tile.TileContext(nc) as tc:
        with (
            tc.tile_pool(name="cst", bufs=1) as cst,
            tc.tile_pool(name="xp", bufs=1) as xp,
            tc.tile_pool(name="wd", bufs=NCH) as wdp,
            tc.tile_pool(name="osb", bufs=3) as osb,
            tc.tile_pool(name="ps", bufs=1, space="PSUM") as psp,
            tc.tile_pool(name="po", bufs=2, space="PSUM") as pop,
        ):
            # --- wgen params first (gpsimd ring; keeps HWDGE rings for x)
            fs_sb = cst.tile([2, NCH * 128], F16, tag="fs")
            nc.gpsimd.dma_start(fs_sb[:], fs_d[:])
            fm_sb = cst.tile([2, NCH * C], F16, tag="fm")
            nc.gpsimd.dma_start(fm_sb[:], fm_d[:])
            mask_sb = cst.tile([128, C], F16, tag="mask")
            nc.gpsimd.dma_start(mask_sb[:], mask_d[:])

            # --- x: 9 paired chunk loads, alternating sync/scalar rings
            xg = xp.tile([128, NCH * R], F16, tag="xg")
            for p in range(9):
                lo = 2 * p * R
                hi = min((2 * p + 2) * R, NCH * R)
                eng = nc.sync if p % 2 == 0 else nc.scalar
                eng.dma_start(xg[:, lo:hi], xg_d[:, lo:hi])

            # --- remaining params
            bias_sb = cst.tile([C, NCH], F32, tag="bias")
            nc.gpsimd.dma_start(bias_sb[:], biasT_d[:])
            uu_sb = cst.tile([128, 16 * 32], F16, tag="uu")
            nc.gpsimd.dma_start(uu_sb[:], uu_d[:])
            predq_sb = cst.tile([128, 32], F16, tag="predq")
            nc.gpsimd.dma_start(predq_sb[:], predq_d[:])
            predb_sb = cst.tile([128, 2], F16, tag="predb")
            nc.gpsimd.dma_start(predb_sb[:], predb_d[:])

            # --- stationary generation: rank-2 with embedded decay rows ---
            wf_sb = []

            def emit_wgen(J):
                pw = pop.tile([128, C], F32, tag="po", name=f"pw{J}")
                nc.tensor.matmul(pw[:], fs_sb[:, 128 * J:128 * (J + 1)],
                                 fm_sb[:, C * J:C * (J + 1)],
                                 start=True, stop=True)
                wf = wdp.tile([128, C], F16, tag="wd", name=f"wd{J}")
                nc.vector.tensor_mul(wf[:], pw[:], mask_sb[:])
                wf_sb.append(wf)

            def emit_main(J, out_sb, col0, eng_dma=None, dma_src=None,
                          dma_dst=None):
                po = pop.tile([C, R], F32, tag="po", name=f"po{J}")
                for h in range(2):
                    nc.tensor.matmul(po[:, 512 * h:512 * (h + 1)], wf_sb[J][:],
                                     xg[:, R * J + 512 * h:
                                        R * J + 512 * (h + 1)],
                                     start=True, stop=True)
                nc.scalar.activation(out_sb[:, col0:col0 + 512],
                                     po[:, 0:512], Ident,
                                     bias=bias_sb[:, J:J + 1])
                nc.vector.tensor_scalar_add(out_sb[:, col0 + 512:col0 + 1024],
                                            po[:, 512:1024],
                                            bias_sb[:, J:J + 1])
                if eng_dma is not None:
                    eng_dma.dma_start(dma_dst, dma_src)

            # chunk 0 has no cross term -> run as soon as x chunk 0 lands
            emit_wgen(0)
            out0 = osb.tile([C, R], F16, tag="os0")
            emit_main(0, out0, 0, nc.sync, out0[:], outg_d[:, 0:R])

            for J in range(1, NCH):
                emit_wgen(J)

            # --- A-phase: 16 col-tiled matmuls per half into 4 PSUM strips
            a_ps = [psp.tile([128, 512], F32, tag=f"pa{h}", name=f"pa{h}")
                    for h in range(2)]

            def emit_a(I):
                g = I % 4
                for h in range(2):
                    nc.tensor.matmul(a_ps[h][32 * g:32 * (g + 1), :],
                                     uu_sb[:, 32 * I:32 * (I + 1)],
                                     xg[:, R * I + 512 * h:
                                        R * I + 512 * (h + 1)],
                                     start=(I <= 3), stop=(I >= NCH - 5),
                                     skip_group_check=True,
                                     tile_position=(0, 32 * g))

            for I in range(NCH - 2):      # chunks 0..14
                emit_a(I)

            # --- pass A: A_1..A_15 are final once chunks 0..14 are in.
            # Emitted BEFORE chunk 15's A matmuls so Tile orders the strip
            # reads first (WAR) and mains 1..15 unblock without waiting for
            # the last chunk.
            a4_sb = cst.tile([128, R], F16, tag="a4")
            a2q_sb = cst.tile([32, R], F16, tag="a2q")
            for h in range(2):
                if h == 0:
                    nc.scalar.activation(a4_sb[:, 0:512], a_ps[0][:], Ident)
                else:
                    nc.vector.tensor_copy(a4_sb[:, 512:1024], a_ps[1][:])
                ar = psp.tile([32, 512], F32, tag=f"pr{h}", name=f"ar{h}")
                nc.tensor.matmul(ar[:], predq_sb[:],
                                 a4_sb[:, 512 * h:512 * (h + 1)],
                                 start=True, stop=True)
                if h == 0:
                    nc.scalar.activation(a2q_sb[:, 0:512], ar[:], Ident)
                else:
                    nc.vector.tensor_copy(a2q_sb[:, 512:1024], ar[:])

            # scatter A_1..A_15 rows into xg partitions 0/1 (2 DMAs; the
            # q-major a2q layout makes src/dst iteration orders line up)
            nc.sync.dma_start(xg[0:1, R:16 * R], a2q_sb[0:15, :])
            nc.gpsimd.dma_start(xg[1:2, R:16 * R], a2q_sb[16:31, :])

            # --- chunk 15's A contribution + pass B for A_16
            emit_a(NCH - 2)
            # A_16 lives on strip rows 0/1 (partitions 32g/32g+1 after the
            # J%16 remap).  Strips 0..2 got their last write at chunks
            # 12/13/14, so pass A's a4_sb copy already holds their final
            # values; chunk 15 only updates strip 3 — refresh partitions
            # 96/97 (32-aligned, same-partition copy) and reduce the four
            # strip rows with a selector stationary.
            a2b_sb = cst.tile([2, R], F16, tag="a2b")
            for h in range(2):
                if h == 0:
                    nc.scalar.activation(a4_sb[96:98, 0:512],
                                         a_ps[0][96:98, :], Ident)
                else:
                    nc.vector.tensor_copy(a4_sb[96:98, 512:1024],
                                          a_ps[1][96:98, :])
                arb = psp.tile([2, 512], F32, tag=f"pr{h}", name=f"arb{h}")
                nc.tensor.matmul(arb[:], predb_sb[:],
                                 a4_sb[:, 512 * h:512 * (h + 1)],
                                 start=True, stop=True)
                if h == 0:
                    nc.scalar.activation(a2b_sb[:, 0:512], arb[:], Ident)
                else:
                    nc.vector.tensor_copy(a2b_sb[:, 512:1024], arb[:])
            nc.gpsimd.dma_start(xg[0:2, 16 * R:17 * R], a2b_sb[:])

            # --- mains: per chunk one K=128 matmul per half + bias copy;
            # outputs flushed as 516KB pairs on sync/gpsimd rings
            for p in range(8):
                J0 = 1 + 2 * p
                pair = osb.tile([C, 2 * R], F16, tag="osb")
                emit_main(J0, pair, 0)
                eng = nc.sync if p % 2 == 0 else nc.gpsimd
                emit_main(J0 + 1, pair, R, eng,
                          pair[:], outg_d[:, R * J0:R * (J0 + 2)])

    nc.compile()
    return nc


def _host_prep(weight, bias, decay_value):
    w0 = np.zeros(C * NCH); w1 = np.zeros(C * NCH)
    w0[:S] = weight[0].astype(np.float64)
    w1[:S] = weight[1].astype(np.float64)
    d0 = float(np.clip(np.float32(decay_value[0, 0]), 0.9, 1.0))
    d1 = float(np.clip(np.float32(decay_value[1, 0]), 0.9, 1.0))
    sl = np.arange(C, dtype=np.float64)

    uu = np.zeros((128, 16 * 32), dtype=np.float16)
    fs = np.zeros((2, NCH * 128), dtype=np.float16)
    fm = np.zeros((2, NCH * C), dtype=np.float16)
    with np.errstate(under='ignore'):
        for I in range(NCH - 1):
            for J in range(I + 1, NCH):
                e = (126.0 * (J - I) - sl) / DC
                m = 2 * (J % 16)        # J=16 -> strip rows 0/1
                uu[2:128, 32 * I + m] = (w0[C * I:C * (I + 1)] * d0 ** e
                                         ).astype(np.float16)
                uu[2:128, 32 * I + m + 1] = (d1 ** e).astype(np.float16)
        for J in range(NCH):
            c0 = C * J
            # stationary factor rows: [p=0] decay row v0, [p=1] v1,
            # [p>=2] diag-block factors (s_loc = p-2)
            if J > 0:
                fs[0, 128 * J + 0] = np.float16(d0 ** (63.0 / DC))
                fs[1, 128 * J + 1] = np.float16(d1 ** (63.0 / DC))
            fs[0, 128 * J + 2:128 * (J + 1)] = (
                w0[c0:c0 + C] * d0 ** ((63.0 - sl) / DC)).astype(np.float16)
            fs[1, 128 * J + 2:128 * (J + 1)] = (
                d1 ** ((63.0 - sl) / DC)).astype(np.float16)
            fm[0, c0:c0 + C] = (d0 ** ((sl - 63.0) / DC)).astype(np.float16)
            fm[1, c0:c0 + C] = (w1[c0:c0 + C] * d1 ** ((sl - 63.0) / DC)
                                ).astype(np.float16)
        fm[:, C * 16 + LAST:] = 0

    mask = np.zeros((128, C), dtype=np.float16)
    mask[0:2, :] = 1
    mask[2:128, :] = (sl[None, :] >= sl[:, None]).astype(np.float16)
    # q-major strip reduce: a2q row 16q+(J-1) = sum_g a4[32g + 2(J%16) + q]
    predq = np.zeros((128, 32), dtype=np.float16)
    for g in range(4):
        for J in range(1, NCH):
            for q in range(2):
                predq[32 * g + 2 * (J % 16) + q, 16 * q + (J - 1)] = 1
    # A_16 selector: sum strip rows 0/1 across the four 32-partition strips
    predb = np.zeros((128, 2), dtype=np.float16)
    for g in range(4):
        for q in range(2):
            predb[32 * g + q, q] = 1
    biasT = np.zeros((C, NCH), dtype=np.float32)
    bias32 = bias.astype(np.float32)
    for J in range(NCH):
        hi = min(C, S - C * J)
        biasT[:hi, J] = bias32[C * J:C * J + hi]
    return uu, fs, fm, mask, predq, predb, biasT


def make_in_maps(inputs):
    x = np.asarray(inputs["x"], dtype=np.float32)
    weight = np.asarray(inputs["weight"], dtype=np.float32)
    bias = np.asarray(inputs["bias"], dtype=np.float32)
    decay_value = np.asarray(inputs["decay_value"], dtype=np.float32)

    uu, fs, fm, mask, predq, predb, biasT = _host_prep(weight, bias,
                                                       decay_value)

    x16 = x.reshape(B * E, S).astype(np.float16)
    in_maps = []
    for c in range(N_CORES):
        xc = x16[R * c:R * (c + 1), :]                    # [R, S]
        xgc = np.zeros((128, NCH * R), dtype=np.float16)  # rows 0/1 zero
        xcT = xc.T                                        # [S, R]
        for J in range(NCH):
            hi = min(C, S - C * J)
            xgc[2:2 + hi, R * J:R * (J + 1)] = xcT[C * J:C * J + hi, :]
        in_maps.append({
            "xg": np.ascontiguousarray(xgc), "uu": uu,
            "fs": fs, "fm": fm, "mask": mask, "predq": predq,
            "predb": predb, "biasT": biasT,
        })
    return in_maps


def kernel(x, weight, bias, decay_value, index=0, recurrent=0, **_):
    global _PROGRAM
    if _PROGRAM is None:
        _PROGRAM = _build_program()
    nc = _PROGRAM

    in_maps = make_in_maps({"x": x, "weight": weight, "bias": bias,
                            "decay_value": decay_value})

    res = run_bass_kernel_spmd(nc, in_maps, core_ids=list(range(N_CORES)))
    out = np.empty((B * E, S), dtype=np.float32)
    for c in range(N_CORES):
        og = np.asarray(res.results[c]["outg"])            # [C, NCH*R] f16
        ot = np.empty((S, R), dtype=np.float32)
        for J in range(NCH):
            hi = min(C, S - C * J)
            ot[C * J:C * J + hi, :] = og[0:hi, R * J:R * (J + 1)
                                         ].astype(np.float32)
        out[R * c:R * (c + 1), :] = ot.T
    return out.reshape(B, E, S)
